# revision 11
# baseline (speedup 1.0000x reference)
"""Trainium2 Bass kernel for nn_MultiHeadAttention_44908178047033.

T5-style MHA (relative-position bias, bidirectional) over
B=2, L=2048, D=768, H=12, DK=64.

Sharding: 8 cores = 2 batches x 4 head-groups (3 heads each).
Each core computes Q/K/V projections for its (batch, 3 heads), fused
transposed-orientation attention (scores kept as S^T [k, q] so the
softmax denominator and the PV contraction both run as PE matmuls
without transposing the probability matrix), and a partial output
projection. Host sums the 4 per-head-group partials per batch.

Relative-position bias: the T5 bias f(k-q) is constant for |k-q| >= 128
(log-bucketing saturates), so
  exp(s + f) = exp(s + cm)            for k-q <= -128  (ACT bias, free)
             = exp(s + cm) * mu       for k-q >= +128  (DVE scalar mult)
             = exp(s + cm + (f - cm)) for |k-q| < 128  (DVE add from a
               host-precomputed per-partition shifted Toeplitz table,
               read with a negative free-dim stride)
"""

import math
import sys
import threading

import numpy as np

sys.path.insert(0, "/opt/trn_rl_repo")

B, L, D = 2, 2048, 768
H, DK = 12, 64
NUM_BUCKETS, MAX_DIST = 32, 128
HP = 3            # heads per core
HD = HP * DK      # 192 cols per head-group
NCORES = 8
KC = 16           # key chunks of 128
NQ = 4            # q slices of 512
CCH = 6           # contraction chunks of 128 over D

_cache = {}
_lock = threading.Lock()


def _np_bucket(d):
    rel = d
    ret = np.zeros_like(rel)
    n = -rel
    nb = NUM_BUCKETS // 2
    ret = ret + (n < 0).astype(np.int32) * nb
    n = np.abs(n)
    mx = nb // 2
    is_small = n < mx
    n_safe = np.maximum(n, 1).astype(np.float32)
    vl = mx + (
        np.log(n_safe / mx) / math.log(MAX_DIST / mx) * (nb - mx)
    ).astype(np.int32)
    vl = np.minimum(vl, nb - 1)
    return ret + np.where(is_small, n, vl)


def _build_program():
    import concourse.bacc as bacc
    import concourse.bass as bass
    import concourse.mybir as mybir
    import concourse.tile as tile

    dt = mybir.dt
    f32, f32r, bf16 = dt.float32, dt.float32r, dt.bfloat16

    nc = bacc.Bacc("TRN2", target_bir_lowering=False, debug=False,
                   num_devices=NCORES)

    qT_d = nc.dram_tensor("qT", [D, L], f32r, kind="ExternalInput").ap()
    kvT_d = nc.dram_tensor("kvT", [D, L], f32r, kind="ExternalInput").ap()
    wq_d = nc.dram_tensor("wq", [D, HD], f32r, kind="ExternalInput").ap()
    wk_d = nc.dram_tensor("wk", [D, HD], f32r, kind="ExternalInput").ap()
    wv_d = nc.dram_tensor("wv", [D, 256], f32r, kind="ExternalInput").ap()
    wo_d = nc.dram_tensor("wo", [64, HP, D], bf16, kind="ExternalInput").ap()
    sh_d = nc.dram_tensor("sh", [HP, 128, 383], f32, kind="ExternalInput").ap()
    msk_d = nc.dram_tensor("msk", [128, KC], f32, kind="ExternalInput").ap()
    cm_d = nc.dram_tensor("cm", [128, HP], f32, kind="ExternalInput").ap()
    cp_d = nc.dram_tensor("cp", [128, HP], f32, kind="ExternalInput").ap()
    out_d = nc.dram_tensor("out_p", [L, D], f32, kind="ExternalOutput").ap()

    with tile.TileContext(nc) as tc:
        with (
            tc.tile_pool(name="const", bufs=1) as cpool,
            tc.tile_pool(name="dyn", bufs=2) as dyn,
            tc.tile_pool(name="nrm", bufs=4) as npool,
            tc.tile_pool(name="sp", bufs=2, space="PSUM") as sp,
            tc.tile_pool(name="pp", bufs=4, space="PSUM") as pp,
        ):
            # ---- persistent SBUF ----
            wq = cpool.tile([128, CCH, HD], f32r, tag="wq")
            wk = cpool.tile([128, CCH, HD], f32r, tag="wk")
            wv = cpool.tile([128, CCH, 256], f32r, tag="wv")
            wo = cpool.tile([64, HP, D], bf16, tag="wo")
            sh = cpool.tile([128, HP, 383], f32, tag="sh")
            msk = cpool.tile([128, KC], f32, tag="msk")
            cmc = cpool.tile([128, HP], f32, tag="cmc")
            cpc = cpool.tile([128, HP], f32, tag="cpc")
            # heads 0,1 stacked on partitions 0-63 / 64-127; head 2 separate
            QTa = cpool.tile([128, L], bf16, tag="QTa")
            QTb = cpool.tile([64, L], bf16, tag="QTb")
            KTa = cpool.tile([128, L], bf16, tag="KTa")
            KTb = cpool.tile([64, L], bf16, tag="KTb")
            Vg = cpool.tile([128, KC, HP, 65], bf16, tag="Vg")
            AT = cpool.tile([64, HP, L], bf16, tag="AT")

            # phase-2 inputs live in the "slab" slots recycled later for
            # probability tiles and output staging
            qT = dyn.tile([128, CCH, L], f32r, tag="slab")
            kvT = dyn.tile([128, CCH, L], f32r, tag="slab")

            # ---- loads (weights first on the scalar HWDGE path so the
            # first projection matmuls can start early; bulk activations
            # stream on sync) ----
            nc.scalar.dma_start(out=wq[:], in_=wq_d.rearrange("(c p) n -> p c n", p=128))
            nc.scalar.dma_start(out=wk[:], in_=wk_d.rearrange("(c p) n -> p c n", p=128))
            nc.scalar.dma_start(out=wv[:], in_=wv_d.rearrange("(c p) n -> p c n", p=128))
            nc.scalar.dma_start(out=wo[:], in_=wo_d)
            nc.scalar.dma_start(out=sh[:], in_=sh_d.rearrange("h p y -> p h y"))
            nc.scalar.dma_start(out=msk[:], in_=msk_d)
            nc.scalar.dma_start(out=cmc[:], in_=cm_d)
            nc.scalar.dma_start(out=cpc[:], in_=cp_d)
            qT_r = qT_d.rearrange("(c p) n -> p c n", p=128)
            kvT_r = kvT_d.rearrange("(c p) n -> p c n", p=128)
            for c in range(CCH):
                for half in range(2):
                    hs = 1024 * half
                    nc.sync.dma_start(out=qT[:, c, hs:hs + 1024],
                                      in_=qT_r[:, c, hs:hs + 1024])
                    nc.sync.dma_start(out=kvT[:, c, hs:hs + 1024],
                                      in_=kvT_r[:, c, hs:hs + 1024])

            # ---- Q/K projections (m-chunks of 128 [heads 0,1] + 64 [head 2]) ----
            for n in range(NQ):
                for (mlo, mw, dstq, dstk) in ((0, 128, QTa, KTa),
                                              (128, 64, QTb, KTb)):
                    ps_q = pp.tile([128, 512], f32, tag="pp", name=f"psq{n}_{mlo}")
                    for c in range(CCH):
                        nc.tensor.matmul(
                            ps_q[0:mw, :],
                            lhsT=wq[:, c, mlo:mlo + mw],
                            rhs=qT[:, c, 512 * n:512 * n + 512],
                            start=(c == 0), stop=(c == CCH - 1),
                        )
                    nc.vector.tensor_copy(
                        dstq[0:mw, 512 * n:512 * n + 512], ps_q[0:mw, :])
                    ps_k = pp.tile([128, 512], f32, tag="pp", name=f"psk{n}_{mlo}")
                    for c in range(CCH):
                        nc.tensor.matmul(
                            ps_k[0:mw, :],
                            lhsT=wk[:, c, mlo:mlo + mw],
                            rhs=kvT[:, c, 512 * n:512 * n + 512],
                            start=(c == 0), stop=(c == CCH - 1),
                        )
                    nc.vector.tensor_copy(
                        dstk[0:mw, 512 * n:512 * n + 512], ps_k[0:mw, :])

            # ---- V projection -> V_aug (bf16) with mask column ----
            for kc in range(KC):
                ps_v = pp.tile([128, 256], f32, tag="pp", name=f"psv{kc}")
                for c in range(CCH):
                    nc.tensor.matmul(
                        ps_v[:],
                        lhsT=kvT[:, c, 128 * kc:128 * kc + 128],
                        rhs=wv[:, c, :],
                        start=(c == 0), stop=(c == CCH - 1),
                    )
                nc.vector.tensor_copy(
                    Vg[:, kc, :, 0:64],
                    ps_v[:, 0:HD].rearrange("p (h d) -> p h d", h=HP))
                mrep = bass.AP(msk[:].tensor, msk[:].offset + kc,
                               [list(msk[:].ap[0]), [0, HP], [1, 1]])
                nc.vector.tensor_copy(Vg[:, kc, :, 64:65], mrep)

            def st_ops(h):
                """(lhsT_base, rhs_base) access helpers for head h."""
                if h == 0:
                    return (lambda kc: KTa[0:64, 128 * kc:128 * kc + 128],
                            lambda lo, w: QTa[0:64, lo:lo + w])
                if h == 1:
                    return (lambda kc: KTa[64:128, 128 * kc:128 * kc + 128],
                            lambda lo, w: QTa[64:128, lo:lo + w])
                return (lambda kc: KTb[0:64, 128 * kc:128 * kc + 128],
                        lambda lo, w: QTb[0:64, lo:lo + w])

            # ---- fused attention, transposed orientation ----
            for h in range(HP):
                kslice, qslice = st_ops(h)
                pvs = [pp.tile([65, 512], f32, tag="pp", name=f"pv{h}_{j}")
                       for j in range(NQ)]
                for kc in range(KC):
                    qlo = max(0, 128 * kc - 128)
                    qhi = min(L, 128 * kc + 255)
                    x0 = (2047 + 128 * kc - qlo) - 1793
                    wcp = max(0, 128 * kc - 128)
                    sh_ap = sh[:, h, :]  # [128, 383]
                    for half in range(2):
                        ha = 1024 * half
                        s = sp.tile([128, 1024], f32, tag="sp",
                                    name=f"s{h}_{kc}_{half}")
                        for jj in range(2):
                            j = 2 * half + jj
                            nc.tensor.matmul(
                                s[:, 512 * jj:512 * jj + 512],
                                lhsT=kslice(kc),
                                rhs=qslice(512 * j, 512),
                                start=True, stop=True,
                            )
                        # near-diagonal bias add (in place, PSUM)
                        a = max(qlo, ha)
                        b = min(qhi, ha + 1024)
                        if b > a:
                            rev = bass.AP(
                                sh_ap.tensor, sh_ap.offset + x0 - (a - qlo),
                                [list(sh_ap.ap[0]), [-1, b - a]],
                            )
                            nc.vector.tensor_add(
                                s[:, a - ha:b - ha], s[:, a - ha:b - ha], rev)
                        # exp with region-split bias: cp for q < wcp, cm after
                        p = dyn.tile([128, 1024], bf16, tag="slab",
                                     name=f"p{h}_{kc}_{half}")
                        wl = min(max(wcp - ha, 0), 1024)
                        if wl > 0:
                            nc.scalar.activation(
                                p[:, 0:wl], s[:, 0:wl],
                                mybir.ActivationFunctionType.Exp,
                                bias=cpc[:, h:h + 1], scale=1.0,
                            )
                        if wl < 1024:
                            nc.scalar.activation(
                                p[:, wl:1024], s[:, wl:1024],
                                mybir.ActivationFunctionType.Exp,
                                bias=cmc[:, h:h + 1], scale=1.0,
                            )
                        for jj in range(2):
                            j = 2 * half + jj
                            nc.tensor.matmul(
                                pvs[j][:],
                                lhsT=Vg[:, kc, h, :],
                                rhs=p[:, 512 * jj:512 * jj + 512],
                                start=(kc == 0), stop=(kc == KC - 1),
                            )
                # evict pv accumulators to SBUF fast so the next head's PV
                # matmuls get the PSUM banks back; normalize off-path
                pvsbs = []
                for j in range(NQ):
                    pvsb = npool.tile([65, 512], f32, tag="pvsb",
                                      name=f"pvsb{h}_{j}")
                    nc.vector.tensor_copy(pvsb[:], pvs[j][:])
                    pvsbs.append(pvsb)
                for j in range(NQ):
                    pvsb = pvsbs[j]
                    inv = npool.tile([1, 512], f32, tag="inv", name=f"i{h}_{j}")
                    nc.vector.reciprocal(inv[:], pvsb[64:65, :])
                    invb = npool.tile([64, 512], f32, tag="invb",
                                      name=f"ib{h}_{j}")
                    nc.gpsimd.partition_broadcast(invb[:], inv[:])
                    nc.vector.tensor_mul(
                        AT[:, h, 512 * j:512 * j + 512], pvsb[0:64, :], invb[:])

            # ---- output projection (partial over this head-group) ----
            for qc in range(L // 128):
                o = dyn.tile([128, D], f32, tag="slab", name=f"o{qc}")
                for nlo, nw in ((0, 512), (512, 256)):
                    ps_o = pp.tile([128, 512], f32, tag="pp", name=f"po{qc}_{nlo}")
                    for h in range(HP):
                        nc.tensor.matmul(
                            ps_o[:, 0:nw],
                            lhsT=AT[:, h, 128 * qc:128 * qc + 128],
                            rhs=wo[:, h, nlo:nlo + nw],
                            start=(h == 0), stop=(h == HP - 1),
                        )
                    nc.vector.tensor_copy(o[:, nlo:nlo + nw], ps_o[:, 0:nw])
                nc.sync.dma_start(
                    out=out_d[128 * qc:128 * qc + 128, :], in_=o[:])

    nc.compile()
    return nc


def _get_program():
    with _lock:
        if "nc" not in _cache:
            _cache["nc"] = _build_program()
        return _cache["nc"]


def _host_prep(core, query, key_value, key_padding_mask, Wq, Wk, Wv, Wo, rel_emb):
    import ml_dtypes

    b, g = core // 4, core % 4
    mask = key_padding_mask[b].astype(np.float32)
    kv = key_value[b] * mask[:, None]
    qT = np.ascontiguousarray(query[b].T)
    kvT = np.ascontiguousarray(kv.T)
    sl = slice(HD * g, HD * (g + 1))
    wq = np.ascontiguousarray(Wq[:, sl])
    wk = np.ascontiguousarray(Wk[:, sl]) * np.float32(DK ** -0.5)
    wv = np.zeros((D, 256), np.float32)
    wv[:, :HD] = Wv[:, sl]
    wo = np.ascontiguousarray(
        Wo[sl].reshape(HP, 64, D).transpose(1, 0, 2)).astype(ml_dtypes.bfloat16)

    d = np.arange(-2047, 2048)
    buckets = _np_bucket(d)
    heads = [HP * g + i for i in range(HP)]
    t = rel_emb[buckets][:, heads].astype(np.float32)  # [4095, HP]
    cm = t[0]
    cp = t[-1]
    # sh[h, p, y] = t[y + 1793 + p, h] - cm[h]
    p_i = np.arange(128)[:, None]
    y_i = np.arange(383)[None, :]
    sh = np.ascontiguousarray(
        (t[y_i + 1793 + p_i] - cm[None, None, :]).transpose(2, 0, 1))
    msk = np.ascontiguousarray(mask.reshape(KC, 128).T)
    cmc = np.ascontiguousarray(np.broadcast_to(cm[None, :], (128, HP)))
    cpc = np.ascontiguousarray(np.broadcast_to(cp[None, :], (128, HP)))
    return {
        "qT": qT, "kvT": kvT, "wq": wq, "wk": wk, "wv": wv, "wo": wo,
        "sh": sh.astype(np.float32), "msk": msk,
        "cm": cmc.astype(np.float32), "cp": cpc.astype(np.float32),
    }


def make_in_maps(**inputs):
    return [_host_prep(c, **inputs) for c in range(NCORES)]


def kernel(query, key_value, key_padding_mask, Wq, Wk, Wv, Wo, rel_emb,
           _results_hook=None, _run_kwargs=None):
    from concourse.bass_utils import run_bass_kernel_spmd

    inputs = dict(query=np.asarray(query), key_value=np.asarray(key_value),
                  key_padding_mask=np.asarray(key_padding_mask),
                  Wq=np.asarray(Wq, np.float32), Wk=np.asarray(Wk, np.float32),
                  Wv=np.asarray(Wv, np.float32), Wo=np.asarray(Wo, np.float32),
                  rel_emb=np.asarray(rel_emb, np.float32))
    nc = _get_program()
    in_maps = make_in_maps(**inputs)
    res = run_bass_kernel_spmd(nc, in_maps, core_ids=list(range(NCORES)),
                               **(_run_kwargs or {}))
    if _results_hook is not None:
        _results_hook(res)
    out = np.zeros((B, L, D), np.float32)
    for c in range(NCORES):
        out[c // 4] += res.results[c]["out_p"]
    return out
